# revision 21
# baseline (speedup 1.0000x reference)
"""Paged-attention GQA decode kernel for Trainium2 (8 NeuronCores, SPMD).

Contract: kernel(**inputs) takes the FULL unsharded inputs of the reference
(q, k, v, k_cache, v_cache, slot_mapping, block_tables, context_lens) and
returns the FULL [NS, NH, HD] float32 output.

Strategy
--------
Work is flattened into uniform 128-token "items" (one sequence x one
128-token span), distributed evenly over the 8 cores; the single SPMD
program is identical on every core and fed per-core index/bias/qT data.

Host side, K and V rows are interleaved into one [65536, 2048] *bf16*
table (the reference's new-token scatter applied to this copy -- slots are
per-sequence disjoint, so semantics are identical).  bf16 halves the HBM
gather traffic, which dominates this kernel, and costs ~0.3% relative
error against the 2e-2 tolerance.

Per item the device:
  1. indirect-DMA-gathers 128 interleaved [K|V] token rows (4KB each);
     padded tokens carry an out-of-bounds index so the bounds check skips
     their fetches entirely (first 5 items fetch a dummy slot instead so
     every kv pool buffer holds finite data before any skip happens),
  2. PE-transposes K per kv-head (bf16 transpose-mode) into one full
     PSUM bank, staged to SBUF with a single big DVE copy; score matmuls
     scores^T[t, qh] = K @ qT use the transposed K as the stationary
     operand (bf16 gets the fast weight-load path; scale folded into qT
     on host),
  3. applies exp AND the valid-token mask in one scalar-engine pass:
     E = exp(scores + bias) with bias 0 / -100 per token partition,
  4. computes the numerator *transposed* with V as the stationary
     operand: numT[d, h] = sum_t V[t, kv(h), d] * E[t, h] -- eight
     [t,128]x[t,4] matmuls into one [128, 32] PSUM tile -- plus
     den = ones^T @ E.  This avoids the baseline's [nh, nkv*hd]
     cross-product (8x the PSUM and output traffic).
  5. stages numT/den into SBUF; one output DMA at the very end.
A ~4.5us warm-up burst of dummy matmuls opens the PE clock-gate while
the first gathers are in flight, and a 2-stage software pipeline
(steady-state PE order T(i) | AV(i-2) | S(i-1)) keeps the in-order
tensor engine from ever waiting on the exp or the staging copy.

Host extracts per-item numT/den, sums partials per sequence, divides.
"""

import math
import os

import numpy as np
import ml_dtypes

from concourse import bacc, bass, mybir
import concourse.tile as tile
from concourse.bass_utils import run_bass_kernel_spmd

N_CORES = 8
TPB = 128          # tokens per work item (= SBUF partitions)
SCALE = 0.08838834764831845     # 1/sqrt(128)

F32 = mybir.dt.float32
BF16 = mybir.dt.bfloat16
I32 = mybir.dt.int32

_prog_cache: dict = {}

LAST_EXEC_NS = None
LAST_RESULTS = None


def _build_program(p2c: int, nslots: int, nkv: int, hd: int, nh: int):
    """One SPMD program processing `p2c` items; per-core behavior is pure data."""
    row = nkv * hd                 # elements per K (or V) token row
    g = nh // nkv                  # GQA group size
    assert hd == TPB, "head_dim must equal 128 for this layout"

    nc = bacc.Bacc("TRN2", target_bir_lowering=False, debug=False)

    kvcat = nc.dram_tensor("kvcat", [nslots, 2 * row], BF16, kind="ExternalInput")
    # qt payload: [qT per item | ones column | 128x128 identity] all bf16
    qt = nc.dram_tensor("qt", [hd, p2c * nh + 1 + TPB], BF16, kind="ExternalInput")
    idx = nc.dram_tensor("idx", [TPB, p2c], I32, kind="ExternalInput")
    bias = nc.dram_tensor("bias", [TPB, p2c], F32, kind="ExternalInput")
    out = nc.dram_tensor("onum", [hd, p2c * nh], F32, kind="ExternalOutput")
    outd = nc.dram_tensor("oden", [1, p2c * nh], F32, kind="ExternalOutput")

    with tile.TileContext(nc) as tc:
        with (
            tc.tile_pool(name="const", bufs=1) as constp,
            tc.tile_pool(name="kv", bufs=5) as kvp,
            tc.tile_pool(name="kt", bufs=2) as ktp,
            tc.tile_pool(name="sm", bufs=2) as smp,
            tc.tile_pool(name="wups", bufs=1, space="PSUM") as wupsp,
            tc.tile_pool(name="ktps", bufs=2, space="PSUM") as ktpsp,
            tc.tile_pool(name="scps", bufs=2, space="PSUM") as scpsp,
            tc.tile_pool(name="ntps", bufs=2, space="PSUM") as ntpsp,
            tc.tile_pool(name="denps", bufs=1, space="PSUM") as denpsp,
        ):
            qt_sb = constp.tile([hd, p2c * nh + 1 + TPB], BF16)
            nc.sync.dma_start(qt_sb[:], qt[:])
            ones_sb = qt_sb[:, p2c * nh: p2c * nh + 1]
            ident = qt_sb[:, p2c * nh + 1: p2c * nh + 1 + TPB]
            idx_sb = constp.tile([TPB, p2c], I32)
            nc.sync.dma_start(idx_sb[:], idx[:])
            bias_sb = constp.tile([TPB, p2c], F32)
            nc.sync.dma_start(bias_sb[:], bias[:])
            nums_sb = constp.tile([hd, p2c * nh], F32)
            dens_sb = constp.tile([1, p2c * nh], F32)

            # HAM warm-up: ~4.5us of back-to-back dummy matmuls while the
            # first gathers are still in flight, so the PE clock-gate opens
            # (1.2 -> 2.4 GHz) before the real work arrives and the
            # transpose-heavy steady state keeps it open
            wu_ps = wupsp.tile([1, 64], F32, tag="wu")
            for _ in range(80):
                nc.tensor.matmul(wu_ps[:], lhsT=ones_sb, rhs=qt_sb[:, :64],
                                 start=True, stop=True)

            def gather(p):
                kv_tile = kvp.tile([TPB, 2 * row], BF16, tag="kv")
                ioff = bass.IndirectOffsetOnAxis(
                    ap=idx_sb[:, p: p + 1], axis=0)
                # padded tokens carry index == nslots (> bounds_check), so
                # their 4KB fetches are skipped entirely; whatever stale
                # data sits in those partitions is killed by the exp bias
                nc.gpsimd.indirect_dma_start(
                    out=kv_tile[:], out_offset=None, in_=kvcat[:],
                    in_offset=ioff, bounds_check=nslots - 1, oob_is_err=False)
                return kv_tile

            def do_T(p, kv_tile):
                # all 8 per-head transposes write one full PSUM bank
                # ([128, 1024] bf16 == 2KB/partition), staged to SBUF with
                # one big DVE copy instead of eight small ones
                kt_ps = ktpsp.tile([TPB, row], BF16, tag="ktp")
                for n in range(nkv):
                    nc.tensor.transpose(
                        kt_ps[:, n * hd:(n + 1) * hd],
                        kv_tile[:, n * hd:(n + 1) * hd], ident)
                kt_sb = ktp.tile([TPB, row], BF16, tag="kt")
                nc.vector.tensor_copy(kt_sb[:], kt_ps[:])
                return kt_sb

            def do_S(p, kt_sb):
                sc_ps = scpsp.tile([TPB, nh], F32, tag="sc")
                for n in range(nkv):
                    nc.tensor.matmul(
                        sc_ps[:, n * g:(n + 1) * g],
                        lhsT=kt_sb[:, n * hd:(n + 1) * hd],
                        rhs=qt_sb[:, p * nh + n * g: p * nh + (n + 1) * g],
                        start=True, stop=True,
                    )
                expT = smp.tile([TPB, nh], BF16, tag="expT")
                # exp AND length-mask in one pass: bias is 0 for valid
                # tokens, -100 for padding (exp(-100) == 0 in bf16)
                nc.scalar.activation(
                    expT[:], sc_ps[:], mybir.ActivationFunctionType.Exp,
                    bias=bias_sb[:, p: p + 1])
                return expT

            def do_av(p, kv_tile, expT):
                nt_ps = ntpsp.tile([hd, nh], F32, tag="nt")
                for n in range(nkv):
                    # numT[d, h] with V as the stationary operand: one
                    # [128, 32] PSUM tile holds the whole per-item output
                    nc.tensor.matmul(
                        nt_ps[:, n * g:(n + 1) * g],
                        lhsT=kv_tile[:, row + n * hd: row + (n + 1) * hd],
                        rhs=expT[:, n * g:(n + 1) * g],
                        start=True, stop=True,
                    )
                den_ps = denpsp.tile([1, nh], F32, tag="den")
                nc.tensor.matmul(
                    den_ps[:], lhsT=ones_sb, rhs=expT[:],
                    start=True, stop=True,
                )
                nc.scalar.activation(
                    nums_sb[:, p * nh:(p + 1) * nh], nt_ps[:],
                    mybir.ActivationFunctionType.Copy)
                nc.scalar.activation(
                    dens_sb[:, p * nh:(p + 1) * nh], den_ps[:],
                    mybir.ActivationFunctionType.Copy)

            # 2-stage software pipeline; steady-state PE order is
            #   T(i) | AV(i-2) | S(i-1)
            # so the kt staging copy of item i and the exp of item i-1 are
            # both covered by ~1.3us of PE work before their consumers
            kvt = [None] * p2c
            kts = [None] * p2c
            exps = [None] * p2c
            kvt[0] = gather(0)
            for i in range(p2c + 2):
                if i + 1 < p2c:
                    kvt[i + 1] = gather(i + 1)
                if i < p2c:
                    kts[i] = do_T(i, kvt[i])
                if i >= 2:
                    do_av(i - 2, kvt[i - 2], exps[i - 2])
                if 0 <= i - 1 < p2c:
                    exps[i - 1] = do_S(i - 1, kts[i - 1])
            nc.sync.dma_start(out[:], nums_sb[:])
            nc.sync.dma_start(outd[:], dens_sb[:])

    nc.compile()
    return nc


def _plan(context_lens: np.ndarray):
    """Flatten (seq, 128-token-block) work items and split them over cores."""
    ns = context_lens.shape[0]
    nblk = [(int(L) + TPB - 1) // TPB for L in context_lens]
    work = [(s, j) for s in range(ns) for j in range(nblk[s])]
    p2c = (len(work) + N_CORES - 1) // N_CORES
    work += [None] * (p2c * N_CORES - len(work))
    per_core = [work[c * p2c:(c + 1) * p2c] for c in range(N_CORES)]
    return p2c, per_core


def _bf16(a: np.ndarray) -> np.ndarray:
    return np.asarray(a, np.float32).astype(ml_dtypes.bfloat16)


def _prepare(q, k, v, k_cache, v_cache, slot_mapping, block_tables, context_lens):
    ns, nh, hd = q.shape
    nb, bs, nkv, _ = k_cache.shape
    nslots = nb * bs
    row = nkv * hd
    g = nh // nkv
    assert hd == TPB and TPB % bs == 0

    # Interleave K and V rows into one [nslots, 2*row] bf16 table so one
    # indirect DMA gathers both, and apply the reference's new-token scatter
    # host-side on this copy (slots are per-sequence disjoint => identical).
    kv = np.empty((nslots, 2 * row), ml_dtypes.bfloat16)
    kv[:, :row] = _bf16(np.ascontiguousarray(k_cache)).reshape(nslots, row)
    kv[:, row:] = _bf16(np.ascontiguousarray(v_cache)).reshape(nslots, row)
    sm = np.asarray(slot_mapping).astype(np.int64)
    kv[sm, :row] = _bf16(k).reshape(ns, row)
    kv[sm, row:] = _bf16(v).reshape(ns, row)

    cl = np.asarray(context_lens).astype(np.int64)
    bt = np.asarray(block_tables).astype(np.int64)
    p2c, per_core = _plan(cl)

    qts, idxs, biases = [], [], []
    for c in range(N_CORES):
        qt_c = np.zeros((hd, p2c * nh + 1 + TPB), ml_dtypes.bfloat16)
        qt_c[:, p2c * nh] = 1.0                                   # ones column
        qt_c[:, p2c * nh + 1:] = np.eye(TPB, dtype=np.float32)    # identity
        # fully-padded items: skip every fetch (index nslots) except in the
        # first 5 items where the kv pool buffers must get initialized
        idx_c = np.broadcast_to(
            np.where(np.arange(p2c) < 5, 0, nslots).astype(np.int32),
            (TPB, p2c)).copy()
        bias_c = np.full((TPB, p2c), -100.0, np.float32)
        for m, item in enumerate(per_core[c]):
            if item is None:
                continue
            s, j = item
            L = int(cl[s])
            nblk = (L + bs - 1) // bs
            qt_c[:, m * nh:(m + 1) * nh] = _bf16(
                np.asarray(q[s], np.float32).T * SCALE)
            t = j * TPB + np.arange(TPB, dtype=np.int64)
            cb = t // bs
            valid = t < L
            # invalid tokens: index `nslots` fails the device bounds check,
            # skipping the fetch.  The first 5 items (= kv pool bufs) fetch
            # dummy slot 0 instead so every pool buffer is fully written
            # with real (finite) data before any fetch is ever skipped.
            pad = 0 if m < 5 else nslots
            slot = np.where(valid, bt[s, np.minimum(cb, nblk - 1)] * bs + t % bs, pad)
            idx_c[:, m] = slot.astype(np.int32)
            bias_c[:, m] = np.where(valid, 0.0, -100.0).astype(np.float32)
        qts.append(qt_c)
        idxs.append(idx_c)
        biases.append(bias_c)

    in_maps = [
        {"kvcat": kv, "qt": qts[c], "idx": idxs[c], "bias": biases[c]}
        for c in range(N_CORES)
    ]
    meta = dict(ns=ns, nh=nh, hd=hd, nkv=nkv, g=g, p2c=p2c, per_core=per_core,
                nslots=nslots)
    return in_maps, meta


def _combine(results, meta):
    ns, nh, hd = meta["ns"], meta["nh"], meta["hd"]
    num = np.zeros((ns, nh, hd), np.float64)
    den = np.zeros((ns, nh), np.float64)
    for c, items in enumerate(meta["per_core"]):
        onum = results[c]["onum"]          # [hd, p2c*nh]
        oden = results[c]["oden"]          # [1, p2c*nh]
        for m, item in enumerate(items):
            if item is None:
                continue
            s, _ = item
            num[s] += onum[:, m * nh:(m + 1) * nh].T
            den[s] += oden[0, m * nh:(m + 1) * nh]
    return (num / den[:, :, None]).astype(np.float32)


def kernel(q, k, v, k_cache, v_cache, slot_mapping, block_tables, context_lens):
    global LAST_EXEC_NS, LAST_RESULTS
    in_maps, meta = _prepare(q, k, v, k_cache, v_cache, slot_mapping,
                             block_tables, context_lens)
    key = (meta["p2c"], meta["nslots"], meta["nkv"], meta["hd"], meta["nh"])
    if key not in _prog_cache:
        _prog_cache[key] = _build_program(*key)
    nc = _prog_cache[key]

    trace = bool(int(os.environ.get("KERNEL_TRACE", "0")))
    res = run_bass_kernel_spmd(nc, in_maps, list(range(N_CORES)), trace=trace)
    LAST_EXEC_NS = res.exec_time_ns
    LAST_RESULTS = res
    return _combine(res.results, meta)


# revision 23
# speedup vs baseline: 1.1590x; 1.1590x over previous
"""Paged-attention GQA decode kernel for Trainium2 (8 NeuronCores, SPMD).

Contract: kernel(**inputs) takes the FULL unsharded inputs of the reference
(q, k, v, k_cache, v_cache, slot_mapping, block_tables, context_lens) and
returns the FULL [NS, NH, HD] float32 output.

Strategy
--------
Work is flattened into uniform 128-token "items" (one sequence x one
128-token span), distributed evenly over the 8 cores; the single SPMD
program is identical on every core and fed per-core index/bias/qT data.

Host side, K and V rows are interleaved into one [65536, 2048] *bf16*
table (the reference's new-token scatter applied to this copy -- slots are
per-sequence disjoint, so semantics are identical).  bf16 halves the HBM
gather traffic, which dominates this kernel, and costs ~0.3% relative
error against the 2e-2 tolerance.

Per item the device:
  1. indirect-DMA-gathers 128 interleaved [K|V] token rows (4KB each);
     padded tokens carry an out-of-bounds index so the bounds check skips
     their fetches entirely (first 5 items fetch a dummy slot instead so
     every kv pool buffer holds finite data before any skip happens),
  2. PE-transposes K per kv-head (bf16 transpose-mode) into one full
     PSUM bank, staged to SBUF with a single big DVE copy; score matmuls
     scores^T[t, qh] = K @ qT use the transposed K as the stationary
     operand (bf16 gets the fast weight-load path; scale folded into qT
     on host),
  3. applies exp AND the valid-token mask in one scalar-engine pass:
     E = exp(scores + bias) with bias 0 / -100 per token partition,
  4. computes the numerator *transposed* with V as the stationary
     operand: numT[d, h] = sum_t V[t, kv(h), d] * E[t, h] -- eight
     [t,128]x[t,4] matmuls into one [128, 32] PSUM tile -- plus
     den = ones^T @ E.  This avoids the baseline's [nh, nkv*hd]
     cross-product (8x the PSUM and output traffic).
  5. stages numT/den into SBUF; one output DMA at the very end.
A ~4.5us warm-up burst of dummy matmuls opens the PE clock-gate while
the first gathers are in flight, and a 2-stage software pipeline
(steady-state PE order T(i) | AV(i-2) | S(i-1)) keeps the in-order
tensor engine from ever waiting on the exp or the staging copy.

Host extracts per-item numT/den, sums partials per sequence, divides.
"""

import math
import os

import numpy as np
import ml_dtypes

from concourse import bacc, bass, mybir
import concourse.tile as tile
from concourse.bass_utils import run_bass_kernel_spmd

N_CORES = 8
TPB = 128          # tokens per work item (= SBUF partitions)
SCALE = 0.08838834764831845     # 1/sqrt(128)

F32 = mybir.dt.float32
BF16 = mybir.dt.bfloat16
I32 = mybir.dt.int32

_prog_cache: dict = {}

LAST_EXEC_NS = None
LAST_RESULTS = None


def _build_program(p2c: int, nslots: int, nkv: int, hd: int, nh: int):
    """One SPMD program processing `p2c` items; per-core behavior is pure data."""
    row = nkv * hd                 # elements per K (or V) token row
    g = nh // nkv                  # GQA group size
    assert hd == TPB, "head_dim must equal 128 for this layout"

    nc = bacc.Bacc("TRN2", target_bir_lowering=False, debug=False)

    kvcat = nc.dram_tensor("kvcat", [nslots, 2 * row], BF16, kind="ExternalInput")
    # qt payload: [qT per item | ones column | 128x128 identity] all bf16
    qt = nc.dram_tensor("qt", [hd, p2c * nh + 1 + TPB], BF16, kind="ExternalInput")
    idx = nc.dram_tensor("idx", [TPB, p2c], I32, kind="ExternalInput")
    bias = nc.dram_tensor("bias", [TPB, p2c], F32, kind="ExternalInput")
    out = nc.dram_tensor("onum", [hd, p2c * nh], F32, kind="ExternalOutput")
    outd = nc.dram_tensor("oden", [1, p2c * nh], F32, kind="ExternalOutput")

    with tile.TileContext(nc) as tc:
        with (
            tc.tile_pool(name="const", bufs=1) as constp,
            tc.tile_pool(name="kv", bufs=5) as kvp,
            tc.tile_pool(name="kt", bufs=2) as ktp,
            tc.tile_pool(name="sm", bufs=2) as smp,
            tc.tile_pool(name="wups", bufs=1, space="PSUM") as wupsp,
            tc.tile_pool(name="ktps", bufs=2, space="PSUM") as ktpsp,
            tc.tile_pool(name="scps", bufs=2, space="PSUM") as scpsp,
            tc.tile_pool(name="ntps", bufs=2, space="PSUM") as ntpsp,
            tc.tile_pool(name="denps", bufs=1, space="PSUM") as denpsp,
        ):
            qt_sb = constp.tile([hd, p2c * nh + 1 + TPB], BF16)
            nc.sync.dma_start(qt_sb[:], qt[:])
            ones_sb = qt_sb[:, p2c * nh: p2c * nh + 1]
            ident = qt_sb[:, p2c * nh + 1: p2c * nh + 1 + TPB]
            idx_sb = constp.tile([TPB, p2c], I32)
            nc.sync.dma_start(idx_sb[:], idx[:])
            bias_sb = constp.tile([TPB, p2c], F32)
            nc.sync.dma_start(bias_sb[:], bias[:])
            nums_sb = constp.tile([hd, p2c * nh], F32)
            dens_sb = constp.tile([1, p2c * nh], F32)

            # HAM warm-up: ~4.5us of back-to-back dummy matmuls while the
            # first gathers are still in flight, so the PE clock-gate opens
            # (1.2 -> 2.4 GHz) before the real work arrives and the
            # transpose-heavy steady state keeps it open
            wu_ps = wupsp.tile([1, 64], F32, tag="wu")
            for _ in range(80):
                nc.tensor.matmul(wu_ps[:], lhsT=ones_sb, rhs=qt_sb[:, :64],
                                 start=True, stop=True)

            def gather(p):
                kv_tile = kvp.tile([TPB, 2 * row], BF16, tag="kv")
                ioff = bass.IndirectOffsetOnAxis(
                    ap=idx_sb[:, p: p + 1], axis=0)
                # padded tokens carry index == nslots (> bounds_check), so
                # their 4KB fetches are skipped entirely; whatever stale
                # data sits in those partitions is killed by the exp bias
                nc.gpsimd.indirect_dma_start(
                    out=kv_tile[:], out_offset=None, in_=kvcat[:],
                    in_offset=ioff, bounds_check=nslots - 1, oob_is_err=False)
                return kv_tile

            def do_T(p, kv_tile):
                # all 8 per-head transposes write one full PSUM bank
                # ([128, 1024] bf16 == 2KB/partition), staged to SBUF with
                # one big DVE copy instead of eight small ones
                kt_ps = ktpsp.tile([TPB, row], BF16, tag="ktp")
                for n in range(nkv):
                    nc.tensor.transpose(
                        kt_ps[:, n * hd:(n + 1) * hd],
                        kv_tile[:, n * hd:(n + 1) * hd], ident)
                kt_sb = ktp.tile([TPB, row], BF16, tag="kt")
                nc.vector.tensor_copy(kt_sb[:], kt_ps[:])
                return kt_sb

            def do_S(p, kt_sb):
                sc_ps = scpsp.tile([TPB, nh], F32, tag="sc")
                for n in range(nkv):
                    nc.tensor.matmul(
                        sc_ps[:, n * g:(n + 1) * g],
                        lhsT=kt_sb[:, n * hd:(n + 1) * hd],
                        rhs=qt_sb[:, p * nh + n * g: p * nh + (n + 1) * g],
                        start=True, stop=True,
                    )
                expT = smp.tile([TPB, nh], BF16, tag="expT")
                # exp AND length-mask in one pass: bias is 0 for valid
                # tokens, -100 for padding (exp(-100) == 0 in bf16)
                nc.scalar.activation(
                    expT[:], sc_ps[:], mybir.ActivationFunctionType.Exp,
                    bias=bias_sb[:, p: p + 1])
                return expT

            def do_av(p, kv_tile, expT):
                nt_ps = ntpsp.tile([hd, nh], F32, tag="nt")
                for n in range(nkv):
                    # numT[d, h] with V as the stationary operand: one
                    # [128, 32] PSUM tile holds the whole per-item output
                    nc.tensor.matmul(
                        nt_ps[:, n * g:(n + 1) * g],
                        lhsT=kv_tile[:, row + n * hd: row + (n + 1) * hd],
                        rhs=expT[:, n * g:(n + 1) * g],
                        start=True, stop=True,
                    )
                den_ps = denpsp.tile([1, nh], F32, tag="den")
                nc.tensor.matmul(
                    den_ps[:], lhsT=ones_sb, rhs=expT[:],
                    start=True, stop=True,
                )
                nc.scalar.activation(
                    nums_sb[:, p * nh:(p + 1) * nh], nt_ps[:],
                    mybir.ActivationFunctionType.Copy)
                nc.scalar.activation(
                    dens_sb[:, p * nh:(p + 1) * nh], den_ps[:],
                    mybir.ActivationFunctionType.Copy)

            # 2-stage software pipeline; steady-state PE order is
            #   T(i) | AV(i-2) | S(i-1)
            # so the kt staging copy of item i and the exp of item i-1 are
            # both covered by ~1.3us of PE work before their consumers
            kvt = [None] * p2c
            kts = [None] * p2c
            exps = [None] * p2c
            kvt[0] = gather(0)
            for i in range(p2c + 2):
                if i + 1 < p2c:
                    kvt[i + 1] = gather(i + 1)
                if i < p2c:
                    kts[i] = do_T(i, kvt[i])
                if i >= 2:
                    do_av(i - 2, kvt[i - 2], exps[i - 2])
                if 0 <= i - 1 < p2c:
                    exps[i - 1] = do_S(i - 1, kts[i - 1])
            nc.sync.dma_start(out[:], nums_sb[:])
            nc.sync.dma_start(outd[:], dens_sb[:])

    nc.compile()
    return nc


def _plan(context_lens: np.ndarray):
    """Flatten (seq, 128-token-block) work items and split them over cores."""
    ns = context_lens.shape[0]
    nblk = [(int(L) + TPB - 1) // TPB for L in context_lens]
    work = [(s, j) for s in range(ns) for j in range(nblk[s])]
    p2c = (len(work) + N_CORES - 1) // N_CORES
    work += [None] * (p2c * N_CORES - len(work))
    per_core = [work[c * p2c:(c + 1) * p2c] for c in range(N_CORES)]
    return p2c, per_core


def _bf16(a: np.ndarray) -> np.ndarray:
    return np.asarray(a, np.float32).astype(ml_dtypes.bfloat16)


def _prepare(q, k, v, k_cache, v_cache, slot_mapping, block_tables, context_lens):
    ns, nh, hd = q.shape
    nb, bs, nkv, _ = k_cache.shape
    nslots = nb * bs
    row = nkv * hd
    g = nh // nkv
    assert hd == TPB and TPB % bs == 0

    # Interleave K and V rows into one [nslots, 2*row] bf16 table so one
    # indirect DMA gathers both, and apply the reference's new-token scatter
    # host-side on this copy (slots are per-sequence disjoint => identical).
    kv = np.empty((nslots, 2 * row), ml_dtypes.bfloat16)
    kv[:, :row] = _bf16(np.ascontiguousarray(k_cache)).reshape(nslots, row)
    kv[:, row:] = _bf16(np.ascontiguousarray(v_cache)).reshape(nslots, row)
    sm = np.asarray(slot_mapping).astype(np.int64)
    kv[sm, :row] = _bf16(k).reshape(ns, row)
    kv[sm, row:] = _bf16(v).reshape(ns, row)

    cl = np.asarray(context_lens).astype(np.int64)
    bt = np.asarray(block_tables).astype(np.int64)
    p2c, per_core = _plan(cl)

    qts, idxs, biases = [], [], []
    for c in range(N_CORES):
        qt_c = np.zeros((hd, p2c * nh + 1 + TPB), ml_dtypes.bfloat16)
        qt_c[:, p2c * nh] = 1.0                                   # ones column
        qt_c[:, p2c * nh + 1:] = np.eye(TPB, dtype=np.float32)    # identity
        # fully-padded items: skip every fetch (index nslots) except in the
        # first 5 items where the kv pool buffers must get initialized
        idx_c = np.broadcast_to(
            np.where(np.arange(p2c) < 5, 0, nslots).astype(np.int32),
            (TPB, p2c)).copy()
        bias_c = np.full((TPB, p2c), -100.0, np.float32)
        for m, item in enumerate(per_core[c]):
            if item is None:
                continue
            s, j = item
            L = int(cl[s])
            nblk = (L + bs - 1) // bs
            qt_c[:, m * nh:(m + 1) * nh] = _bf16(
                np.asarray(q[s], np.float32).T * SCALE)
            t = j * TPB + np.arange(TPB, dtype=np.int64)
            cb = t // bs
            valid = t < L
            # invalid tokens: index `nslots` fails the device bounds check,
            # skipping the fetch.  The first 5 items (= kv pool bufs) fetch
            # dummy slot 0 instead so every pool buffer is fully written
            # with real (finite) data before any fetch is ever skipped.
            pad = 0 if m < 5 else nslots
            slot = np.where(valid, bt[s, np.minimum(cb, nblk - 1)] * bs + t % bs, pad)
            idx_c[:, m] = slot.astype(np.int32)
            bias_c[:, m] = np.where(valid, 0.0, -100.0).astype(np.float32)
        qts.append(qt_c)
        idxs.append(idx_c)
        biases.append(bias_c)

    in_maps = [
        {"kvcat": kv, "qt": qts[c], "idx": idxs[c], "bias": biases[c]}
        for c in range(N_CORES)
    ]
    meta = dict(ns=ns, nh=nh, hd=hd, nkv=nkv, g=g, p2c=p2c, per_core=per_core,
                nslots=nslots)
    return in_maps, meta


def _combine(results, meta):
    ns, nh, hd = meta["ns"], meta["nh"], meta["hd"]
    num = np.zeros((ns, nh, hd), np.float64)
    den = np.zeros((ns, nh), np.float64)
    for c, items in enumerate(meta["per_core"]):
        onum = results[c]["onum"]          # [hd, p2c*nh]
        oden = results[c]["oden"]          # [1, p2c*nh]
        for m, item in enumerate(items):
            if item is None:
                continue
            s, _ = item
            num[s] += onum[:, m * nh:(m + 1) * nh].T
            den[s] += oden[0, m * nh:(m + 1) * nh]
    return (num / den[:, :, None]).astype(np.float32)


def kernel(q, k, v, k_cache, v_cache, slot_mapping, block_tables, context_lens):
    global LAST_EXEC_NS, LAST_RESULTS
    in_maps, meta = _prepare(q, k, v, k_cache, v_cache, slot_mapping,
                             block_tables, context_lens)
    key = (meta["p2c"], meta["nslots"], meta["nkv"], meta["hd"], meta["nh"])
    if key not in _prog_cache:
        _prog_cache[key] = _build_program(*key)
    nc = _prog_cache[key]

    trace = bool(int(os.environ.get("KERNEL_TRACE", "0")))
    res = run_bass_kernel_spmd(nc, in_maps, list(range(N_CORES)), trace=trace)
    LAST_EXEC_NS = res.exec_time_ns
    LAST_RESULTS = res
    return _combine(res.results, meta)
